# revision 10
# baseline (speedup 1.0000x reference)
"""Trainium2 Bass kernel for the 2-layer minLSTM problem (B=16, T=2048,
A=128, E=H=M=512), data-parallel over batch across 8 NeuronCores (2 rows
per core, no collectives).

Math (exact rewrites of the reference):
  - gates: with d = softplus(-f)-softplus(-i): f_gate = sigmoid(-d)
    = sigmoid(f)/(sigmoid(f)+sigmoid(i)); i_gate = 1 - f_gate.
  - g(x) = where(x>=0, x+0.5, sigmoid(x)) = relu(x) + min(sigmoid(x), 0.5)
  - scan: h_t = f_gate_t*h_{t-1} + i_gate_t*g_t, h_0 = 1 — a convex
    combination, numerically stable in linear space; identical to the
    reference's log-space parallel scan. Runs on the native
    tensor_tensor_scan instruction (fp32 state) along the free dim.
  - layer-0 pre-acts: emb[x] @ W == onehot(x) @ (emb @ W); EW on host.
  - last-valid-step gather: sum_t h1[:,t]*mask[t] with a host-built onehot
    mask over T (mask row zeroed + output offset 1.0 when lengths==0).

Layout: activations live as (128 channels, T) tiles — 4 channel blocks per
row. Matmuls (bf16 in / fp32 PSUM accum) produce gate pre-activations
directly in this layout, the scan consumes it, and layer-1 matmuls consume
the scan output with no transposes anywhere.
"""
import os
import sys
import json

for _p in ("/opt/trn_rl_repo", "/root/.axon_site/_ro/trn_rl_repo",
           "/root/.axon_site/_ro/pypackages"):
    if os.path.isdir(_p) and _p not in sys.path:
        sys.path.append(_p)

import numpy as np
import ml_dtypes
import concourse.bass as bass
import concourse.tile as tile
from concourse import mybir
from concourse.tile import add_dep_helper

fp32 = mybir.dt.float32
fp32r = mybir.dt.float32r
bf16 = mybir.dt.bfloat16

B, T, A, E, H, M = 16, 2048, 128, 512, 512, 512
N_CORES = 8
ROWS = B // N_CORES  # batch rows per core
HB = H // 128        # 4 channel blocks
TC = 512             # time chunk (= one fp32 PSUM bank)


def _i(r):
    return getattr(r, "ins", r)


def _act_recip(nc, out, in_):
    """ACT-table reciprocal. bass bans the helper over far-range accuracy;
    operands here are sigmoid sums in [~0.2, 2] where the table is accurate
    (HW-measured ~4e-6 rel in this range)."""
    imm = lambda v: mybir.ImmediateValue(dtype=mybir.dt.float32, value=v)
    return nc.scalar.add_instruction(
        mybir.InstActivation(
            name=nc.get_next_instruction_name(),
            func=mybir.ActivationFunctionType.Reciprocal,
            ins=[nc.scalar.lower_ap(in_), imm(0.0), imm(1.0), imm(0.0)],
            outs=[nc.scalar.lower_ap(out)],
        )
    )


def _col(src):
    """1-D AP (n,) -> 2-D (n, 1)."""
    return bass.AP(tensor=src.tensor, offset=src.offset,
                   ap=[list(src.ap[0]), [0, 1]])


def _row(src):
    """1-D AP (n,) -> 2-D (1, n)."""
    return bass.AP(tensor=src.tensor, offset=src.offset,
                   ap=[[0, 1], list(src.ap[0])])


def _bcast128(src2d):
    """(1, n) AP -> (128, n) with partition stride 0."""
    return bass.AP(tensor=src2d.tensor, offset=src2d.offset,
                   ap=[[0, 128]] + [list(a) for a in src2d.ap[1:]])


def _split_waits(bir: dict, max_waits: int = 1) -> int:
    """This container's walrus supports one sync-wait slot per instruction;
    move excess on_wait entries onto preceding NoOps (same engine — the
    sequencer stalls at the NoOp, semantics preserved)."""
    n = 0
    for f in bir.get("functions", []):
        for bb in f.get("blocks", []):
            out = []
            for inst in bb.get("instructions", []):
                si = inst.get("sync_info")
                ow = list((si or {}).get("on_wait") or [])
                if si is not None and len(ow) > max_waits:
                    extra, keep = ow[:-max_waits], ow[-max_waits:]
                    for j in range(0, len(extra), max_waits):
                        out.append({
                            "debug": inst.get("debug", 0),
                            "engine": inst["engine"],
                            "ins": [], "outs": [],
                            "name": f"{inst['name']}-wsplit{j}",
                            "opcode": "NoOp",
                            "sync_info": {"on_update": [],
                                          "on_wait": extra[j:j + max_waits]},
                        })
                        n += 1
                    si["on_wait"] = keep
                out.append(inst)
            bb["instructions"] = out
    return n


def _install_birfix(nc):
    orig = nc.to_json_bytes

    def patched():
        d = json.loads(orig())
        _split_waits(d, max_waits=1)
        return json.dumps(d).encode()

    nc.to_json_bytes = patched


def build_nc(t_len=T):
    """Per-core Bass program (SPMD: same program on all 8 cores)."""
    nc = bass.Bass("TRN2", target_bir_lowering=False)
    ntc = t_len // TC
    AF = mybir.ActivationFunctionType
    OP = mybir.AluOpType

    oh = nc.declare_dram_parameter("oh", [ROWS, 128, t_len], bf16, isOutput=False)
    ew = nc.declare_dram_parameter("ew", [3, 128, H], bf16, isOutput=False)
    w1 = nc.declare_dram_parameter("w1", [3, H, H], bf16, isOutput=False)
    b0 = nc.declare_dram_parameter("b0", [3, H], fp32, isOutput=False)
    b1 = nc.declare_dram_parameter("b1", [3, H], fp32, isOutput=False)
    wm0 = nc.declare_dram_parameter("wm0", [H, M], fp32r, isOutput=False)
    wm1 = nc.declare_dram_parameter("wm1", [M, M], fp32r, isOutput=False)
    wout = nc.declare_dram_parameter("wout", [M, 1], fp32r, isOutput=False)
    bm0 = nc.declare_dram_parameter("bm0", [M], fp32, isOutput=False)
    bm1 = nc.declare_dram_parameter("bm1", [M], fp32, isOutput=False)
    bout = nc.declare_dram_parameter("bout", [1], fp32, isOutput=False)
    mask = nc.declare_dram_parameter("mask", [ROWS, t_len], bf16, isOutput=False)
    ofs = nc.declare_dram_parameter("ofs", [ROWS], fp32, isOutput=False)
    out = nc.declare_dram_parameter("out", [ROWS], fp32, isOutput=True)

    with tile.TileContext(nc) as tc:
        with tc.tile_pool(name="wts", bufs=1) as wts, \
             tc.tile_pool(name="bias", bufs=1) as bias, \
             tc.tile_pool(name="h0p", bufs=1) as h0p, \
             tc.tile_pool(name="work", bufs=2) as work, \
             tc.tile_pool(name="boundary", bufs=2) as bnd, \
             tc.tile_pool(name="accs", bufs=1) as accp, \
             tc.tile_pool(name="mlp", bufs=1) as mlpp, \
             tc.tile_pool(name="ps", bufs=2, space="PSUM") as ps, \
             tc.tile_pool(name="psm", bufs=1, space="PSUM") as psm:

            # ---- resident loads -------------------------------------------
            ewt = []
            for g in range(3):
                t = wts.tile([128, H], bf16, tag=f"ew{g}")
                nc.sync.dma_start(out=t, in_=ew[g])
                ewt.append(t)
            w1t = [[None] * HB for _ in range(3)]
            for g in range(3):
                for kb in range(HB):
                    t = wts.tile([128, H], bf16, tag=f"w1_{g}_{kb}")
                    nc.sync.dma_start(out=t, in_=w1[g, kb * 128:(kb + 1) * 128, :])
                    w1t[g][kb] = t
            oht = []
            for r in range(ROWS):
                t = wts.tile([128, t_len], bf16, tag=f"oh{r}")
                nc.sync.dma_start(out=t, in_=oh[r])
                oht.append(t)
            maskt = []
            for r in range(ROWS):
                t = wts.tile([128, t_len], bf16, tag=f"mask{r}")
                nc.sync.dma_start(out=t, in_=_bcast128(mask[r:r + 1, :]))
                maskt.append(t)
            bt_l = [[[None] * HB for _ in range(3)] for _ in range(2)]
            for li, bsrc in enumerate((b0, b1)):
                for g in range(3):
                    for hb in range(HB):
                        t = bias.tile([128, 1], fp32, tag=f"b{li}_{g}_{hb}")
                        nc.sync.dma_start(
                            out=t, in_=_col(bsrc[g, hb * 128:(hb + 1) * 128]))
                        bt_l[li][g][hb] = t
            bm0t, bm1t = [], []
            for mo in range(HB):
                t = bias.tile([128, 1], fp32, tag=f"bm0_{mo}")
                nc.sync.dma_start(out=t, in_=_col(bm0[mo * 128:(mo + 1) * 128]))
                bm0t.append(t)
                t = bias.tile([128, 1], fp32, tag=f"bm1_{mo}")
                nc.sync.dma_start(out=t, in_=_col(bm1[mo * 128:(mo + 1) * 128]))
                bm1t.append(t)
            boutt = bias.tile([1, 1], fp32, tag="bout")
            nc.sync.dma_start(out=boutt, in_=_col(bout[0:1]))
            ofst = bias.tile([128, ROWS], fp32, tag="ofs")
            nc.sync.dma_start(out=ofst, in_=_bcast128(_row(ofs[0:ROWS])))

            # ---- recurrent layers -----------------------------------------
            h_prev = None                 # layer-0 outputs, per (r, hb)
            value2 = [None] * HB          # (128, ROWS) selected states
            last_act = None               # ACT-order chain (table sets)

            for layer in range(2):
                h_cur = [[None] * HB for _ in range(ROWS)]
                for r in range(ROWS):
                    for hb in range(HB):
                        bt = bt_l[layer]
                        F = bnd.tile([128, t_len], bf16, tag="F")
                        g_ = bnd.tile([128, t_len], bf16, tag="g_")
                        S = bnd.tile([128, t_len], bf16, tag="S")
                        rl = bnd.tile([128, t_len], bf16, tag="rl")
                        q = bnd.tile([128, t_len], bf16, tag="q")
                        I = bnd.tile([128, t_len], bf16, tag="I")
                        rq = bnd.tile([128, t_len], bf16, tag="rq")
                        sig_insts = []
                        for tcn in range(ntc):
                            sl = slice(tcn * TC, (tcn + 1) * TC)
                            pg = []
                            for g in range(3):
                                p = ps.tile([128, TC], fp32, tag=f"ps{g}")
                                if layer == 0:
                                    nc.tensor.matmul(
                                        p, ewt[g][:, hb * 128:(hb + 1) * 128],
                                        oht[r][:, sl], start=True, stop=True)
                                else:
                                    for kb in range(HB):
                                        nc.tensor.matmul(
                                            p, w1t[g][kb][:, hb * 128:(hb + 1) * 128],
                                            h_prev[r][kb][:, sl],
                                            start=(kb == 0), stop=(kb == HB - 1))
                                pg.append(p)
                            pass
                            s0 = _i(nc.scalar.activation(
                                out=F[:, sl], in_=pg[0], func=AF.Sigmoid,
                                bias=bt[0][hb], scale=1.0))
                            s1 = _i(nc.scalar.activation(
                                out=I[:, sl], in_=pg[1], func=AF.Sigmoid,
                                bias=bt[1][hb], scale=1.0))
                            s2 = _i(nc.scalar.activation(
                                out=S[:, sl], in_=pg[2], func=AF.Sigmoid,
                                bias=bt[2][hb], scale=1.0))
                            sig_insts += [s0, s1, s2]
                            if last_act is not None:
                                add_dep_helper(s0, last_act, False,
                                               "ACT set order")
                            # relu(th + bh) straight from PSUM on DVE
                            nc.vector.tensor_scalar(
                                rl[:, sl], pg[2], bt[2][hb], 0.0,
                                OP.add, OP.max)
                        nc.vector.tensor_add(q, F, I)
                        # one full-row reciprocal per unit (fewer ACT
                        # instructions and table switches)
                        ri = _i(_act_recip(nc, rq, q))
                        add_dep_helper(ri, sig_insts[-1], False,
                                       "ACT set order")
                        last_act = ri

                        # full-row gate algebra (bf16 2x where additive-only)
                        fg = bnd.tile([128, t_len], bf16, tag="fg")
                        nc.vector.tensor_mul(fg, F, rq)
                        ig = work.tile([128, t_len], bf16, tag="ig")
                        nc.gpsimd.tensor_scalar(ig, fg, -1.0, 1.0,
                                                OP.mult, OP.add)
                        nc.vector.scalar_tensor_tensor(
                            g_, S, 0.5, rl, OP.min, OP.add)
                        bb = work.tile([128, t_len], bf16, tag="bb")
                        nc.gpsimd.tensor_mul(bb, ig, g_)
                        if layer == 0:
                            h = h0p.tile([128, t_len], bf16, tag=f"h0_{r}_{hb}")
                            nc.vector.tensor_tensor_scan(
                                h, fg, bb, 1.0, OP.mult, OP.add)
                            h_cur[r][hb] = h
                        else:
                            h1 = bnd.tile([128, t_len], bf16, tag="h1", bufs=1)
                            nc.vector.tensor_tensor_scan(
                                h1, fg, bb, 1.0, OP.mult, OP.add)
                            if value2[hb] is None:
                                value2[hb] = mlpp.tile(
                                    [128, ROWS], fp32r,
                                    name=f"val{hb}", tag=f"val{hb}")
                            # fused select: acc = sum_t h1*mask  (scratch
                            # output reuses the dead fg slot)
                            scr = bnd.tile([128, t_len], bf16, tag="fg")
                            vsum = work.tile([128, 1], fp32, tag="vsum")
                            nc.vector.scalar_tensor_tensor(
                                scr, h1, 1.0, maskt[r], OP.mult, OP.mult,
                                accum_out=vsum)
                            nc.vector.tensor_tensor(
                                value2[hb][:, r:r + 1], vsum,
                                ofst[:, r:r + 1], OP.add)
                if layer == 0:
                    h_prev = h_cur

            # ---- MLP head --------------------------------------------------
            cur = value2
            for wmt_d, bmt in ((wm0, bm0t), (wm1, bm1t)):
                wtiles = []
                for kb in range(HB):
                    t = mlpp.tile([128, M], fp32r, tag=f"wm_{kb}")
                    nc.sync.dma_start(out=t, in_=wmt_d[kb * 128:(kb + 1) * 128, :])
                    wtiles.append(t)
                nxt = []
                for mo in range(HB):
                    p = psm.tile([128, ROWS], fp32, tag="mlpps")
                    for kb in range(HB):
                        nc.tensor.matmul(p, wtiles[kb][:, mo * 128:(mo + 1) * 128],
                                         cur[kb], start=(kb == 0),
                                         stop=(kb == HB - 1))
                    o = mlpp.tile([128, ROWS], fp32r, tag=f"mlp_o{mo}",
                                  bufs=2)
                    nc.scalar.activation(out=o, in_=p, func=AF.Relu,
                                         bias=bmt[mo], scale=1.0)
                    nxt.append(o)
                cur = nxt
            # W_out: (512,1) loaded as (128, HB), column kb = block kb
            wo = mlpp.tile([128, HB], fp32r, tag="wo")
            wsrc = wout[:, :]
            nc.sync.dma_start(out=wo, in_=bass.AP(
                tensor=wsrc.tensor, offset=wsrc.offset,
                ap=[[1, 128], [128, HB]]))
            pfin = psm.tile([1, ROWS], fp32, tag="finps")
            for kb in range(HB):
                nc.tensor.matmul(pfin, wo[:, kb:kb + 1], cur[kb],
                                 start=(kb == 0), stop=(kb == HB - 1))
            fin = mlpp.tile([1, ROWS], fp32, tag="fin")
            nc.scalar.activation(out=fin, in_=pfin, func=AF.Sigmoid,
                                 bias=boutt, scale=1.0)
            nc.sync.dma_start(out=_row(out[0:ROWS]), in_=fin)

    _install_birfix(nc)
    return nc


def prep_inputs(x, lengths, emb, Wf0, bf0, Wi0, bi0, Wh0, bh0,
                Wf1, bf1, Wi1, bi1, Wh1, bh1,
                W_mlp0, b_mlp0, W_mlp1, b_mlp1, W_out, b_out, t_len=T):
    """Host-side prep: one-hot encode x, fold emb into the layer-0 weights,
    build selection masks. Returns per-core input maps."""
    f32 = np.float32
    b16 = ml_dtypes.bfloat16
    x = np.asarray(x).astype(np.int64)
    lengths = np.asarray(lengths).astype(np.int64)
    emb = np.asarray(emb, f32)

    ew = np.stack([emb @ np.asarray(w, f32) for w in (Wf0, Wi0, Wh0)])
    b0 = np.stack([np.asarray(b, f32) for b in (bf0, bi0, bh0)])
    w1 = np.stack([np.asarray(w, f32) for w in (Wf1, Wi1, Wh1)])
    b1 = np.stack([np.asarray(b, f32) for b in (bf1, bi1, bh1)])

    rows_b = x.shape[0]
    onehot = np.zeros((rows_b, A, t_len), f32)
    bi_, ti_ = np.meshgrid(np.arange(rows_b), np.arange(t_len), indexing="ij")
    onehot[bi_.ravel(), x.ravel(), ti_.ravel()] = 1.0

    idx = np.minimum(np.maximum(lengths - 1, 0), t_len - 1)
    mask = np.zeros((rows_b, t_len), f32)
    mask[np.arange(rows_b), idx] = 1.0
    mask[lengths == 0] = 0.0
    ofs = (lengths == 0).astype(f32)

    common = dict(
        ew=np.ascontiguousarray(ew.astype(b16)),
        w1=np.ascontiguousarray(w1.astype(b16)),
        b0=np.ascontiguousarray(b0), b1=np.ascontiguousarray(b1),
        wm0=np.asarray(W_mlp0, f32), wm1=np.asarray(W_mlp1, f32),
        wout=np.asarray(W_out, f32),
        bm0=np.asarray(b_mlp0, f32), bm1=np.asarray(b_mlp1, f32),
        bout=np.asarray(b_out, f32),
    )
    in_maps = []
    n_cores = rows_b // ROWS
    for c in range(n_cores):
        sl = slice(c * ROWS, (c + 1) * ROWS)
        m = dict(common)
        m["oh"] = np.ascontiguousarray(onehot[sl].astype(b16))
        m["mask"] = np.ascontiguousarray(mask[sl].astype(b16))
        m["ofs"] = np.ascontiguousarray(ofs[sl])
        in_maps.append(m)
    return in_maps


_NC_CACHE = {}


def kernel(**inputs) -> np.ndarray:
    from concourse.bass_utils import run_bass_kernel_spmd
    if T not in _NC_CACHE:
        _NC_CACHE[T] = build_nc(T)
    nc = _NC_CACHE[T]
    in_maps = prep_inputs(**inputs)
    res = run_bass_kernel_spmd(nc, in_maps, list(range(N_CORES)))
    outs = [np.asarray(res.results[c]["out"], np.float32).reshape(ROWS)
            for c in range(N_CORES)]
    return np.concatenate(outs)


# revision 13
# speedup vs baseline: 1.1189x; 1.1189x over previous
"""Trainium2 Bass kernel for the 2-layer minLSTM problem (B=16, T=2048,
A=128, E=H=M=512), data-parallel over batch across 8 NeuronCores (2 rows
per core, no collectives).

Math (exact rewrites of the reference):
  - gates: with d = softplus(-f)-softplus(-i): f_gate = sigmoid(-d)
    = sigmoid(f)/(sigmoid(f)+sigmoid(i)); i_gate = 1 - f_gate.
  - g(x) = where(x>=0, x+0.5, sigmoid(x)) = relu(x) + min(sigmoid(x), 0.5)
  - scan: h_t = f_gate_t*h_{t-1} + i_gate_t*g_t, h_0 = 1 — a convex
    combination, numerically stable in linear space; identical to the
    reference's log-space parallel scan. Runs on the native
    tensor_tensor_scan instruction (fp32 state) along the free dim.
  - layer-0 pre-acts: emb[x] @ W == onehot(x) @ (emb @ W); EW on host.
  - last-valid-step gather: sum_t h1[:,t]*mask[t] with a host-built onehot
    mask over T (mask row zeroed + output offset 1.0 when lengths==0).

Layout: activations live as (128 channels, T) tiles — 4 channel blocks per
row. Matmuls (bf16 in / fp32 PSUM accum) produce gate pre-activations
directly in this layout, the scan consumes it, and layer-1 matmuls consume
the scan output with no transposes anywhere.
"""
import os
import sys
import json

for _p in ("/opt/trn_rl_repo", "/root/.axon_site/_ro/trn_rl_repo",
           "/root/.axon_site/_ro/pypackages"):
    if os.path.isdir(_p) and _p not in sys.path:
        sys.path.append(_p)

import numpy as np
import ml_dtypes
import concourse.bass as bass
import concourse.tile as tile
from concourse import mybir
from concourse.tile import add_dep_helper

fp32 = mybir.dt.float32
fp32r = mybir.dt.float32r
bf16 = mybir.dt.bfloat16

B, T, A, E, H, M = 16, 2048, 128, 512, 512, 512
N_CORES = 8
ROWS = B // N_CORES  # batch rows per core
HB = H // 128        # 4 channel blocks
TC = 512             # time chunk (= one fp32 PSUM bank)


def _i(r):
    return getattr(r, "ins", r)


def _act_recip(nc, out, in_):
    """ACT-table reciprocal. bass bans the helper over far-range accuracy;
    operands here are sigmoid sums in [~0.2, 2] where the table is accurate
    (HW-measured ~4e-6 rel in this range)."""
    imm = lambda v: mybir.ImmediateValue(dtype=mybir.dt.float32, value=v)
    return nc.scalar.add_instruction(
        mybir.InstActivation(
            name=nc.get_next_instruction_name(),
            func=mybir.ActivationFunctionType.Reciprocal,
            ins=[nc.scalar.lower_ap(in_), imm(0.0), imm(1.0), imm(0.0)],
            outs=[nc.scalar.lower_ap(out)],
        )
    )


def _col(src):
    """1-D AP (n,) -> 2-D (n, 1)."""
    return bass.AP(tensor=src.tensor, offset=src.offset,
                   ap=[list(src.ap[0]), [0, 1]])


def _row(src):
    """1-D AP (n,) -> 2-D (1, n)."""
    return bass.AP(tensor=src.tensor, offset=src.offset,
                   ap=[[0, 1], list(src.ap[0])])


def _bcast128(src2d):
    """(1, n) AP -> (128, n) with partition stride 0."""
    return bass.AP(tensor=src2d.tensor, offset=src2d.offset,
                   ap=[[0, 128]] + [list(a) for a in src2d.ap[1:]])


def _split_waits(bir: dict, max_waits: int = 1) -> int:
    """This container's walrus supports one sync-wait slot per instruction;
    move excess on_wait entries onto preceding NoOps (same engine — the
    sequencer stalls at the NoOp, semantics preserved)."""
    n = 0
    for f in bir.get("functions", []):
        for bb in f.get("blocks", []):
            out = []
            for inst in bb.get("instructions", []):
                si = inst.get("sync_info")
                ow = list((si or {}).get("on_wait") or [])
                if si is not None and len(ow) > max_waits:
                    extra, keep = ow[:-max_waits], ow[-max_waits:]
                    for j in range(0, len(extra), max_waits):
                        out.append({
                            "debug": inst.get("debug", 0),
                            "engine": inst["engine"],
                            "ins": [], "outs": [],
                            "name": f"{inst['name']}-wsplit{j}",
                            "opcode": "NoOp",
                            "sync_info": {"on_update": [],
                                          "on_wait": extra[j:j + max_waits]},
                        })
                        n += 1
                    si["on_wait"] = keep
                out.append(inst)
            bb["instructions"] = out
    return n


def _install_birfix(nc):
    orig = nc.to_json_bytes

    def patched():
        d = json.loads(orig())
        _split_waits(d, max_waits=1)
        return json.dumps(d).encode()

    nc.to_json_bytes = patched


def build_nc(t_len=T):
    """Per-core Bass program (SPMD: same program on all 8 cores)."""
    nc = bass.Bass("TRN2", target_bir_lowering=False)
    ntc = t_len // TC
    AF = mybir.ActivationFunctionType
    OP = mybir.AluOpType

    oh = nc.declare_dram_parameter("oh", [ROWS, 128, t_len], bf16, isOutput=False)
    ew = nc.declare_dram_parameter("ew", [3, 128, H], bf16, isOutput=False)
    w1 = nc.declare_dram_parameter("w1", [3, H, H], bf16, isOutput=False)
    b0 = nc.declare_dram_parameter("b0", [3, H], fp32, isOutput=False)
    b1 = nc.declare_dram_parameter("b1", [3, H], fp32, isOutput=False)
    wm0 = nc.declare_dram_parameter("wm0", [H, M], fp32r, isOutput=False)
    wm1 = nc.declare_dram_parameter("wm1", [M, M], fp32r, isOutput=False)
    wout = nc.declare_dram_parameter("wout", [M, 1], fp32r, isOutput=False)
    bm0 = nc.declare_dram_parameter("bm0", [M], fp32, isOutput=False)
    bm1 = nc.declare_dram_parameter("bm1", [M], fp32, isOutput=False)
    bout = nc.declare_dram_parameter("bout", [1], fp32, isOutput=False)
    mask = nc.declare_dram_parameter("mask", [ROWS, t_len], bf16, isOutput=False)
    ofs = nc.declare_dram_parameter("ofs", [ROWS], fp32, isOutput=False)
    out = nc.declare_dram_parameter("out", [ROWS], fp32, isOutput=True)

    with tile.TileContext(nc) as tc:
        with tc.tile_pool(name="wts", bufs=1) as wts, \
             tc.tile_pool(name="bias", bufs=1) as bias, \
             tc.tile_pool(name="h0p", bufs=1) as h0p, \
             tc.tile_pool(name="work", bufs=2) as work, \
             tc.tile_pool(name="boundary", bufs=2) as bnd, \
             tc.tile_pool(name="accs", bufs=1) as accp, \
             tc.tile_pool(name="mlp", bufs=1) as mlpp, \
             tc.tile_pool(name="ps", bufs=2, space="PSUM") as ps, \
             tc.tile_pool(name="psm", bufs=1, space="PSUM") as psm:

            # ---- resident loads -------------------------------------------
            ewt = []
            for g in range(3):
                t = wts.tile([128, H], bf16, tag=f"ew{g}")
                nc.sync.dma_start(out=t, in_=ew[g])
                ewt.append(t)
            w1t = [[None] * HB for _ in range(3)]
            for g in range(3):
                for kb in range(HB):
                    t = wts.tile([128, H], bf16, tag=f"w1_{g}_{kb}")
                    nc.sync.dma_start(out=t, in_=w1[g, kb * 128:(kb + 1) * 128, :])
                    w1t[g][kb] = t
            oht = []
            for r in range(ROWS):
                t = wts.tile([128, t_len], bf16, tag=f"oh{r}")
                nc.sync.dma_start(out=t, in_=oh[r])
                oht.append(t)
            maskt = []
            for r in range(ROWS):
                t = wts.tile([128, t_len], bf16, tag=f"mask{r}")
                nc.sync.dma_start(out=t, in_=_bcast128(mask[r:r + 1, :]))
                maskt.append(t)
            bt_l = [[[None] * HB for _ in range(3)] for _ in range(2)]
            for li, bsrc in enumerate((b0, b1)):
                for g in range(3):
                    for hb in range(HB):
                        t = bias.tile([128, 1], fp32, tag=f"b{li}_{g}_{hb}")
                        nc.sync.dma_start(
                            out=t, in_=_col(bsrc[g, hb * 128:(hb + 1) * 128]))
                        bt_l[li][g][hb] = t
            bm0t, bm1t = [], []
            for mo in range(HB):
                t = bias.tile([128, 1], fp32, tag=f"bm0_{mo}")
                nc.sync.dma_start(out=t, in_=_col(bm0[mo * 128:(mo + 1) * 128]))
                bm0t.append(t)
                t = bias.tile([128, 1], fp32, tag=f"bm1_{mo}")
                nc.sync.dma_start(out=t, in_=_col(bm1[mo * 128:(mo + 1) * 128]))
                bm1t.append(t)
            boutt = bias.tile([1, 1], fp32, tag="bout")
            nc.sync.dma_start(out=boutt, in_=_col(bout[0:1]))
            ofst = bias.tile([128, ROWS], fp32, tag="ofs")
            nc.sync.dma_start(out=ofst, in_=_bcast128(_row(ofs[0:ROWS])))

            # ---- recurrent layers -----------------------------------------
            h_prev = None                 # layer-0 outputs, per (r, hb)
            value2 = [None] * HB          # (128, ROWS) selected states
            last_act = None               # ACT-order chain (table sets)

            for layer in range(2):
                h_cur = [[None] * HB for _ in range(ROWS)]
                TL2 = ROWS * t_len
                for hb in range(HB):
                    bt = bt_l[layer]
                    F = bnd.tile([128, TL2], bf16, tag="F")
                    g_ = bnd.tile([128, TL2], bf16, tag="g_")
                    S = bnd.tile([128, TL2], bf16, tag="S", bufs=1)
                    rl = bnd.tile([128, TL2], bf16, tag="rl", bufs=1)
                    q = bnd.tile([128, TL2], bf16, tag="q", bufs=1)
                    rq = bnd.tile([128, TL2], bf16, tag="rq", bufs=1)
                    sig_insts = []
                    for r in range(ROWS):
                        for tcn in range(ntc):
                            sl = slice(r * t_len + tcn * TC,
                                       r * t_len + (tcn + 1) * TC)
                            slr = slice(tcn * TC, (tcn + 1) * TC)
                            pg = []
                            for g in range(3):
                                p = ps.tile([128, TC], fp32, tag=f"ps{g}")
                                if layer == 0:
                                    nc.tensor.matmul(
                                        p, ewt[g][:, hb * 128:(hb + 1) * 128],
                                        oht[r][:, slr], start=True, stop=True)
                                else:
                                    for kb in range(HB):
                                        nc.tensor.matmul(
                                            p, w1t[g][kb][:, hb * 128:(hb + 1) * 128],
                                            h_prev[r][kb][:, slr],
                                            start=(kb == 0), stop=(kb == HB - 1))
                                pg.append(p)
                            s0 = _i(nc.scalar.activation(
                                out=F[:, sl], in_=pg[0], func=AF.Sigmoid,
                                bias=bt[0][hb], scale=1.0))
                            I = work.tile([128, TC], bf16, tag="I")
                            s1 = _i(nc.scalar.activation(
                                out=I, in_=pg[1], func=AF.Sigmoid,
                                bias=bt[1][hb], scale=1.0))
                            s2 = _i(nc.scalar.activation(
                                out=S[:, sl], in_=pg[2], func=AF.Sigmoid,
                                bias=bt[2][hb], scale=1.0))
                            sig_insts += [s0, s1, s2]
                            if last_act is not None:
                                add_dep_helper(s0, last_act, False,
                                               "ACT set order")
                            # relu(th + bh) straight from PSUM on DVE
                            nc.vector.tensor_scalar(
                                rl[:, sl], pg[2], bt[2][hb], 0.0,
                                OP.add, OP.max)
                            nc.vector.tensor_add(q[:, sl], F[:, sl], I)
                    ri = _i(_act_recip(nc, rq, q))
                    add_dep_helper(ri, sig_insts[-1], False, "ACT set order")
                    last_act = ri

                    fg = bnd.tile([128, TL2], bf16, tag="fg")
                    nc.vector.tensor_mul(fg, F, rq)
                    ig = work.tile([128, TL2], bf16, tag="ig")
                    nc.vector.tensor_scalar(ig, fg, -1.0, 1.0,
                                            OP.mult, OP.add)
                    nc.vector.scalar_tensor_tensor(
                        g_, S, 0.5, rl, OP.min, OP.add)
                    bb = work.tile([128, TL2], bf16, tag="bb")
                    nc.vector.tensor_mul(bb, ig, g_)
                    if layer == 0:
                        h = h0p.tile([128, TL2], bf16, tag=f"h0_{hb}")
                        for r in range(ROWS):
                            rsl = slice(r * t_len, (r + 1) * t_len)
                            nc.vector.tensor_tensor_scan(
                                h[:, rsl], fg[:, rsl], bb[:, rsl], 1.0,
                                OP.mult, OP.add)
                        for r in range(ROWS):
                            h_cur[r][hb] = h[:, r * t_len:(r + 1) * t_len]
                    else:
                        h1 = bnd.tile([128, TL2], bf16, tag="h1", bufs=1)
                        if value2[hb] is None:
                            value2[hb] = mlpp.tile(
                                [128, ROWS], fp32r,
                                name=f"val{hb}", tag=f"val{hb}")
                        scr = bnd.tile([128, TL2], bf16, tag="fg")
                        for r in range(ROWS):
                            rsl = slice(r * t_len, (r + 1) * t_len)
                            nc.vector.tensor_tensor_scan(
                                h1[:, rsl], fg[:, rsl], bb[:, rsl], 1.0,
                                OP.mult, OP.add)
                            vsum = work.tile([128, 1], fp32, tag="vsum")
                            nc.vector.scalar_tensor_tensor(
                                scr[:, rsl], h1[:, rsl], 1.0, maskt[r],
                                OP.mult, OP.mult, accum_out=vsum)
                            nc.vector.tensor_tensor(
                                value2[hb][:, r:r + 1], vsum,
                                ofst[:, r:r + 1], OP.add)
                if layer == 0:
                    h_prev = h_cur

            # ---- MLP head --------------------------------------------------
            cur = value2
            for wmt_d, bmt in ((wm0, bm0t), (wm1, bm1t)):
                wtiles = []
                for kb in range(HB):
                    t = mlpp.tile([128, M], fp32r, tag=f"wm_{kb}")
                    nc.sync.dma_start(out=t, in_=wmt_d[kb * 128:(kb + 1) * 128, :])
                    wtiles.append(t)
                nxt = []
                for mo in range(HB):
                    p = psm.tile([128, ROWS], fp32, tag="mlpps")
                    for kb in range(HB):
                        nc.tensor.matmul(p, wtiles[kb][:, mo * 128:(mo + 1) * 128],
                                         cur[kb], start=(kb == 0),
                                         stop=(kb == HB - 1))
                    o = mlpp.tile([128, ROWS], fp32r, tag=f"mlp_o{mo}",
                                  bufs=2)
                    nc.scalar.activation(out=o, in_=p, func=AF.Relu,
                                         bias=bmt[mo], scale=1.0)
                    nxt.append(o)
                cur = nxt
            # W_out: (512,1) loaded as (128, HB), column kb = block kb
            wo = mlpp.tile([128, HB], fp32r, tag="wo")
            wsrc = wout[:, :]
            nc.sync.dma_start(out=wo, in_=bass.AP(
                tensor=wsrc.tensor, offset=wsrc.offset,
                ap=[[1, 128], [128, HB]]))
            pfin = psm.tile([1, ROWS], fp32, tag="finps")
            for kb in range(HB):
                nc.tensor.matmul(pfin, wo[:, kb:kb + 1], cur[kb],
                                 start=(kb == 0), stop=(kb == HB - 1))
            fin = mlpp.tile([1, ROWS], fp32, tag="fin")
            nc.scalar.activation(out=fin, in_=pfin, func=AF.Sigmoid,
                                 bias=boutt, scale=1.0)
            nc.sync.dma_start(out=_row(out[0:ROWS]), in_=fin)

    _install_birfix(nc)
    return nc


def prep_inputs(x, lengths, emb, Wf0, bf0, Wi0, bi0, Wh0, bh0,
                Wf1, bf1, Wi1, bi1, Wh1, bh1,
                W_mlp0, b_mlp0, W_mlp1, b_mlp1, W_out, b_out, t_len=T):
    """Host-side prep: one-hot encode x, fold emb into the layer-0 weights,
    build selection masks. Returns per-core input maps."""
    f32 = np.float32
    b16 = ml_dtypes.bfloat16
    x = np.asarray(x).astype(np.int64)
    lengths = np.asarray(lengths).astype(np.int64)
    emb = np.asarray(emb, f32)

    ew = np.stack([emb @ np.asarray(w, f32) for w in (Wf0, Wi0, Wh0)])
    b0 = np.stack([np.asarray(b, f32) for b in (bf0, bi0, bh0)])
    w1 = np.stack([np.asarray(w, f32) for w in (Wf1, Wi1, Wh1)])
    b1 = np.stack([np.asarray(b, f32) for b in (bf1, bi1, bh1)])

    rows_b = x.shape[0]
    onehot = np.zeros((rows_b, A, t_len), f32)
    bi_, ti_ = np.meshgrid(np.arange(rows_b), np.arange(t_len), indexing="ij")
    onehot[bi_.ravel(), x.ravel(), ti_.ravel()] = 1.0

    idx = np.minimum(np.maximum(lengths - 1, 0), t_len - 1)
    mask = np.zeros((rows_b, t_len), f32)
    mask[np.arange(rows_b), idx] = 1.0
    mask[lengths == 0] = 0.0
    ofs = (lengths == 0).astype(f32)

    common = dict(
        ew=np.ascontiguousarray(ew.astype(b16)),
        w1=np.ascontiguousarray(w1.astype(b16)),
        b0=np.ascontiguousarray(b0), b1=np.ascontiguousarray(b1),
        wm0=np.asarray(W_mlp0, f32), wm1=np.asarray(W_mlp1, f32),
        wout=np.asarray(W_out, f32),
        bm0=np.asarray(b_mlp0, f32), bm1=np.asarray(b_mlp1, f32),
        bout=np.asarray(b_out, f32),
    )
    in_maps = []
    n_cores = rows_b // ROWS
    for c in range(n_cores):
        sl = slice(c * ROWS, (c + 1) * ROWS)
        m = dict(common)
        m["oh"] = np.ascontiguousarray(onehot[sl].astype(b16))
        m["mask"] = np.ascontiguousarray(mask[sl].astype(b16))
        m["ofs"] = np.ascontiguousarray(ofs[sl])
        in_maps.append(m)
    return in_maps


_NC_CACHE = {}


def kernel(**inputs) -> np.ndarray:
    from concourse.bass_utils import run_bass_kernel_spmd
    if T not in _NC_CACHE:
        _NC_CACHE[T] = build_nc(T)
    nc = _NC_CACHE[T]
    in_maps = prep_inputs(**inputs)
    res = run_bass_kernel_spmd(nc, in_maps, list(range(N_CORES)))
    outs = [np.asarray(res.results[c]["out"], np.float32).reshape(ROWS)
            for c in range(N_CORES)]
    return np.concatenate(outs)


# revision 14
# speedup vs baseline: 1.1518x; 1.0294x over previous
"""Trainium2 Bass kernel for the 2-layer minLSTM problem (B=16, T=2048,
A=128, E=H=M=512), data-parallel over batch across 8 NeuronCores (2 rows
per core, no collectives).

Math (exact rewrites of the reference):
  - gates: with d = softplus(-f)-softplus(-i): f_gate = sigmoid(-d)
    = sigmoid(f)/(sigmoid(f)+sigmoid(i)); i_gate = 1 - f_gate.
  - g(x) = where(x>=0, x+0.5, sigmoid(x)) = relu(x) + min(sigmoid(x), 0.5)
  - scan: h_t = f_gate_t*h_{t-1} + i_gate_t*g_t, h_0 = 1 — a convex
    combination, numerically stable in linear space; identical to the
    reference's log-space parallel scan. Runs on the native
    tensor_tensor_scan instruction (fp32 state) along the free dim.
  - layer-0 pre-acts: emb[x] @ W == onehot(x) @ (emb @ W); EW on host.
  - last-valid-step gather: sum_t h1[:,t]*mask[t] with a host-built onehot
    mask over T (mask row zeroed + output offset 1.0 when lengths==0).

Layout: activations live as (128 channels, T) tiles — 4 channel blocks per
row. Matmuls (bf16 in / fp32 PSUM accum) produce gate pre-activations
directly in this layout, the scan consumes it, and layer-1 matmuls consume
the scan output with no transposes anywhere.
"""
import os
import sys
import json

for _p in ("/opt/trn_rl_repo", "/root/.axon_site/_ro/trn_rl_repo",
           "/root/.axon_site/_ro/pypackages"):
    if os.path.isdir(_p) and _p not in sys.path:
        sys.path.append(_p)

import numpy as np
import ml_dtypes
import concourse.bass as bass
import concourse.tile as tile
from concourse import mybir
from concourse.tile import add_dep_helper

fp32 = mybir.dt.float32
fp32r = mybir.dt.float32r
bf16 = mybir.dt.bfloat16

B, T, A, E, H, M = 16, 2048, 128, 512, 512, 512
N_CORES = 8
ROWS = B // N_CORES  # batch rows per core
HB = H // 128        # 4 channel blocks
TC = 512             # time chunk (= one fp32 PSUM bank)


def _i(r):
    return getattr(r, "ins", r)


def _act_recip(nc, out, in_):
    """ACT-table reciprocal. bass bans the helper over far-range accuracy;
    operands here are sigmoid sums in [~0.2, 2] where the table is accurate
    (HW-measured ~4e-6 rel in this range)."""
    imm = lambda v: mybir.ImmediateValue(dtype=mybir.dt.float32, value=v)
    return nc.scalar.add_instruction(
        mybir.InstActivation(
            name=nc.get_next_instruction_name(),
            func=mybir.ActivationFunctionType.Reciprocal,
            ins=[nc.scalar.lower_ap(in_), imm(0.0), imm(1.0), imm(0.0)],
            outs=[nc.scalar.lower_ap(out)],
        )
    )


def _col(src):
    """1-D AP (n,) -> 2-D (n, 1)."""
    return bass.AP(tensor=src.tensor, offset=src.offset,
                   ap=[list(src.ap[0]), [0, 1]])


def _row(src):
    """1-D AP (n,) -> 2-D (1, n)."""
    return bass.AP(tensor=src.tensor, offset=src.offset,
                   ap=[[0, 1], list(src.ap[0])])


def _bcast128(src2d):
    """(1, n) AP -> (128, n) with partition stride 0."""
    return bass.AP(tensor=src2d.tensor, offset=src2d.offset,
                   ap=[[0, 128]] + [list(a) for a in src2d.ap[1:]])


def _split_waits(bir: dict, max_waits: int = 1) -> int:
    """This container's walrus supports one sync-wait slot per instruction;
    move excess on_wait entries onto preceding NoOps (same engine — the
    sequencer stalls at the NoOp, semantics preserved)."""
    n = 0
    for f in bir.get("functions", []):
        for bb in f.get("blocks", []):
            out = []
            for inst in bb.get("instructions", []):
                si = inst.get("sync_info")
                ow = list((si or {}).get("on_wait") or [])
                if si is not None and len(ow) > max_waits:
                    extra, keep = ow[:-max_waits], ow[-max_waits:]
                    for j in range(0, len(extra), max_waits):
                        out.append({
                            "debug": inst.get("debug", 0),
                            "engine": inst["engine"],
                            "ins": [], "outs": [],
                            "name": f"{inst['name']}-wsplit{j}",
                            "opcode": "NoOp",
                            "sync_info": {"on_update": [],
                                          "on_wait": extra[j:j + max_waits]},
                        })
                        n += 1
                    si["on_wait"] = keep
                out.append(inst)
            bb["instructions"] = out
    return n


def _install_birfix(nc):
    orig = nc.to_json_bytes

    def patched():
        d = json.loads(orig())
        _split_waits(d, max_waits=1)
        return json.dumps(d).encode()

    nc.to_json_bytes = patched


def build_nc(t_len=T):
    """Per-core Bass program (SPMD: same program on all 8 cores)."""
    nc = bass.Bass("TRN2", target_bir_lowering=False)
    ntc = t_len // TC
    AF = mybir.ActivationFunctionType
    OP = mybir.AluOpType

    oh = nc.declare_dram_parameter("oh", [ROWS, 128, t_len], bf16, isOutput=False)
    ew = nc.declare_dram_parameter("ew", [3, 128, H], bf16, isOutput=False)
    w1 = nc.declare_dram_parameter("w1", [3, H, H], bf16, isOutput=False)
    b0 = nc.declare_dram_parameter("b0", [3, H], fp32, isOutput=False)
    b1 = nc.declare_dram_parameter("b1", [3, H], fp32, isOutput=False)
    wm0 = nc.declare_dram_parameter("wm0", [H, M], fp32r, isOutput=False)
    wm1 = nc.declare_dram_parameter("wm1", [M, M], fp32r, isOutput=False)
    wout = nc.declare_dram_parameter("wout", [M, 1], fp32r, isOutput=False)
    bm0 = nc.declare_dram_parameter("bm0", [M], fp32, isOutput=False)
    bm1 = nc.declare_dram_parameter("bm1", [M], fp32, isOutput=False)
    bout = nc.declare_dram_parameter("bout", [1], fp32, isOutput=False)
    mask = nc.declare_dram_parameter("mask", [ROWS, t_len], bf16, isOutput=False)
    ofs = nc.declare_dram_parameter("ofs", [ROWS], fp32, isOutput=False)
    out = nc.declare_dram_parameter("out", [ROWS], fp32, isOutput=True)

    with tile.TileContext(nc) as tc:
        with tc.tile_pool(name="wts", bufs=1) as wts, \
             tc.tile_pool(name="bias", bufs=1) as bias, \
             tc.tile_pool(name="h0p", bufs=1) as h0p, \
             tc.tile_pool(name="work", bufs=2) as work, \
             tc.tile_pool(name="boundary", bufs=2) as bnd, \
             tc.tile_pool(name="accs", bufs=1) as accp, \
             tc.tile_pool(name="mlp", bufs=1) as mlpp, \
             tc.tile_pool(name="ps", bufs=2, space="PSUM") as ps, \
             tc.tile_pool(name="psm", bufs=1, space="PSUM") as psm:

            # ---- resident loads -------------------------------------------
            ewt = []
            for g in range(3):
                t = wts.tile([128, H], bf16, tag=f"ew{g}")
                nc.sync.dma_start(out=t, in_=ew[g])
                ewt.append(t)
            w1t = [[None] * HB for _ in range(3)]
            for g in range(3):
                for kb in range(HB):
                    t = wts.tile([128, H], bf16, tag=f"w1_{g}_{kb}")
                    nc.sync.dma_start(out=t, in_=w1[g, kb * 128:(kb + 1) * 128, :])
                    w1t[g][kb] = t
            oht = []
            for r in range(ROWS):
                t = wts.tile([128, t_len], bf16, tag=f"oh{r}")
                nc.sync.dma_start(out=t, in_=oh[r])
                oht.append(t)
            maskt = []
            for r in range(ROWS):
                t = wts.tile([128, t_len], bf16, tag=f"mask{r}")
                nc.sync.dma_start(out=t, in_=_bcast128(mask[r:r + 1, :]))
                maskt.append(t)
            bt_l = [[[None] * HB for _ in range(3)] for _ in range(2)]
            for li, bsrc in enumerate((b0, b1)):
                for g in range(3):
                    for hb in range(HB):
                        t = bias.tile([128, 1], fp32, tag=f"b{li}_{g}_{hb}")
                        nc.sync.dma_start(
                            out=t, in_=_col(bsrc[g, hb * 128:(hb + 1) * 128]))
                        bt_l[li][g][hb] = t
            bm0t, bm1t = [], []
            for mo in range(HB):
                t = bias.tile([128, 1], fp32, tag=f"bm0_{mo}")
                nc.sync.dma_start(out=t, in_=_col(bm0[mo * 128:(mo + 1) * 128]))
                bm0t.append(t)
                t = bias.tile([128, 1], fp32, tag=f"bm1_{mo}")
                nc.sync.dma_start(out=t, in_=_col(bm1[mo * 128:(mo + 1) * 128]))
                bm1t.append(t)
            boutt = bias.tile([1, 1], fp32, tag="bout")
            nc.sync.dma_start(out=boutt, in_=_col(bout[0:1]))
            ofst = bias.tile([128, ROWS], fp32, tag="ofs")
            nc.sync.dma_start(out=ofst, in_=_bcast128(_row(ofs[0:ROWS])))

            # ---- recurrent layers -----------------------------------------
            h_prev = None                 # layer-0 outputs, per (r, hb)
            value2 = [None] * HB          # (128, ROWS) selected states
            last_act = None               # ACT-order chain (table sets)

            for layer in range(2):
                h_cur = [[None] * HB for _ in range(ROWS)]
                for r in range(ROWS):
                    for hb in range(HB):
                        bt = bt_l[layer]
                        F = bnd.tile([128, t_len], bf16, tag="F")
                        g_ = bnd.tile([128, t_len], bf16, tag="g_")
                        S = bnd.tile([128, t_len], bf16, tag="S")
                        rl = bnd.tile([128, t_len], bf16, tag="rl")
                        q = bnd.tile([128, t_len], bf16, tag="q")
                        rq = bnd.tile([128, t_len], bf16, tag="rq")
                        sig_insts = []
                        for tcn in range(ntc):
                            sl = slice(tcn * TC, (tcn + 1) * TC)
                            pg = []
                            for g in range(3):
                                p = ps.tile([128, TC], fp32, tag=f"ps{g}")
                                if layer == 0:
                                    nc.tensor.matmul(
                                        p, ewt[g][:, hb * 128:(hb + 1) * 128],
                                        oht[r][:, sl], start=True, stop=True)
                                else:
                                    for kb in range(HB):
                                        nc.tensor.matmul(
                                            p, w1t[g][kb][:, hb * 128:(hb + 1) * 128],
                                            h_prev[r][kb][:, sl],
                                            start=(kb == 0), stop=(kb == HB - 1))
                                pg.append(p)
                            I = work.tile([128, TC], bf16, tag="I")
                            s0 = _i(nc.scalar.activation(
                                out=F[:, sl], in_=pg[0], func=AF.Sigmoid,
                                bias=bt[0][hb], scale=1.0))
                            s1 = _i(nc.scalar.activation(
                                out=I, in_=pg[1], func=AF.Sigmoid,
                                bias=bt[1][hb], scale=1.0))
                            s2 = _i(nc.scalar.activation(
                                out=S[:, sl], in_=pg[2], func=AF.Sigmoid,
                                bias=bt[2][hb], scale=1.0))
                            sig_insts += [s0, s1, s2]
                            if last_act is not None:
                                add_dep_helper(s0, last_act, False,
                                               "ACT set order")
                            # relu(th + bh) straight from PSUM on DVE
                            nc.vector.tensor_scalar(
                                rl[:, sl], pg[2], bt[2][hb], 0.0,
                                OP.add, OP.max)
                            nc.vector.tensor_add(q[:, sl], F[:, sl], I)
                        # one full-row reciprocal per unit (fewer ACT
                        # instructions and table switches)
                        ri = _i(_act_recip(nc, rq, q))
                        add_dep_helper(ri, sig_insts[-1], False,
                                       "ACT set order")
                        last_act = ri

                        # full-row gate algebra (bf16 2x where additive-only)
                        fg = bnd.tile([128, t_len], bf16, tag="fg")
                        nc.vector.tensor_mul(fg, F, rq)
                        ig = work.tile([128, t_len], bf16, tag="ig")
                        nc.vector.tensor_scalar(ig, fg, -1.0, 1.0,
                                                OP.mult, OP.add)
                        nc.vector.scalar_tensor_tensor(
                            g_, S, 0.5, rl, OP.min, OP.add)
                        bb = work.tile([128, t_len], bf16, tag="bb")
                        nc.vector.tensor_mul(bb, ig, g_)
                        if layer == 0:
                            h = h0p.tile([128, t_len], bf16, tag=f"h0_{r}_{hb}")
                            nc.vector.tensor_tensor_scan(
                                h, fg, bb, 1.0, OP.mult, OP.add)
                            h_cur[r][hb] = h
                        else:
                            h1 = bnd.tile([128, t_len], bf16, tag="h1", bufs=1)
                            nc.vector.tensor_tensor_scan(
                                h1, fg, bb, 1.0, OP.mult, OP.add)
                            if value2[hb] is None:
                                value2[hb] = mlpp.tile(
                                    [128, ROWS], fp32r,
                                    name=f"val{hb}", tag=f"val{hb}")
                            # fused select: acc = sum_t h1*mask  (scratch
                            # output reuses the dead fg slot)
                            scr = bnd.tile([128, t_len], bf16, tag="fg")
                            vsum = work.tile([128, 1], fp32, tag="vsum")
                            nc.vector.scalar_tensor_tensor(
                                scr, h1, 1.0, maskt[r], OP.mult, OP.mult,
                                accum_out=vsum)
                            nc.vector.tensor_tensor(
                                value2[hb][:, r:r + 1], vsum,
                                ofst[:, r:r + 1], OP.add)
                if layer == 0:
                    h_prev = h_cur

            # ---- MLP head --------------------------------------------------
            cur = value2
            for wmt_d, bmt in ((wm0, bm0t), (wm1, bm1t)):
                wtiles = []
                for kb in range(HB):
                    t = mlpp.tile([128, M], fp32r, tag=f"wm_{kb}")
                    nc.sync.dma_start(out=t, in_=wmt_d[kb * 128:(kb + 1) * 128, :])
                    wtiles.append(t)
                nxt = []
                for mo in range(HB):
                    p = psm.tile([128, ROWS], fp32, tag="mlpps")
                    for kb in range(HB):
                        nc.tensor.matmul(p, wtiles[kb][:, mo * 128:(mo + 1) * 128],
                                         cur[kb], start=(kb == 0),
                                         stop=(kb == HB - 1))
                    o = mlpp.tile([128, ROWS], fp32r, tag=f"mlp_o{mo}",
                                  bufs=2)
                    nc.scalar.activation(out=o, in_=p, func=AF.Relu,
                                         bias=bmt[mo], scale=1.0)
                    nxt.append(o)
                cur = nxt
            # W_out: (512,1) loaded as (128, HB), column kb = block kb
            wo = mlpp.tile([128, HB], fp32r, tag="wo")
            wsrc = wout[:, :]
            nc.sync.dma_start(out=wo, in_=bass.AP(
                tensor=wsrc.tensor, offset=wsrc.offset,
                ap=[[1, 128], [128, HB]]))
            pfin = psm.tile([1, ROWS], fp32, tag="finps")
            for kb in range(HB):
                nc.tensor.matmul(pfin, wo[:, kb:kb + 1], cur[kb],
                                 start=(kb == 0), stop=(kb == HB - 1))
            fin = mlpp.tile([1, ROWS], fp32, tag="fin")
            nc.scalar.activation(out=fin, in_=pfin, func=AF.Sigmoid,
                                 bias=boutt, scale=1.0)
            nc.sync.dma_start(out=_row(out[0:ROWS]), in_=fin)

    _install_birfix(nc)
    return nc


def prep_inputs(x, lengths, emb, Wf0, bf0, Wi0, bi0, Wh0, bh0,
                Wf1, bf1, Wi1, bi1, Wh1, bh1,
                W_mlp0, b_mlp0, W_mlp1, b_mlp1, W_out, b_out, t_len=T):
    """Host-side prep: one-hot encode x, fold emb into the layer-0 weights,
    build selection masks. Returns per-core input maps."""
    f32 = np.float32
    b16 = ml_dtypes.bfloat16
    x = np.asarray(x).astype(np.int64)
    lengths = np.asarray(lengths).astype(np.int64)
    emb = np.asarray(emb, f32)

    ew = np.stack([emb @ np.asarray(w, f32) for w in (Wf0, Wi0, Wh0)])
    b0 = np.stack([np.asarray(b, f32) for b in (bf0, bi0, bh0)])
    w1 = np.stack([np.asarray(w, f32) for w in (Wf1, Wi1, Wh1)])
    b1 = np.stack([np.asarray(b, f32) for b in (bf1, bi1, bh1)])

    rows_b = x.shape[0]
    onehot = np.zeros((rows_b, A, t_len), f32)
    bi_, ti_ = np.meshgrid(np.arange(rows_b), np.arange(t_len), indexing="ij")
    onehot[bi_.ravel(), x.ravel(), ti_.ravel()] = 1.0

    idx = np.minimum(np.maximum(lengths - 1, 0), t_len - 1)
    mask = np.zeros((rows_b, t_len), f32)
    mask[np.arange(rows_b), idx] = 1.0
    mask[lengths == 0] = 0.0
    ofs = (lengths == 0).astype(f32)

    common = dict(
        ew=np.ascontiguousarray(ew.astype(b16)),
        w1=np.ascontiguousarray(w1.astype(b16)),
        b0=np.ascontiguousarray(b0), b1=np.ascontiguousarray(b1),
        wm0=np.asarray(W_mlp0, f32), wm1=np.asarray(W_mlp1, f32),
        wout=np.asarray(W_out, f32),
        bm0=np.asarray(b_mlp0, f32), bm1=np.asarray(b_mlp1, f32),
        bout=np.asarray(b_out, f32),
    )
    in_maps = []
    n_cores = rows_b // ROWS
    for c in range(n_cores):
        sl = slice(c * ROWS, (c + 1) * ROWS)
        m = dict(common)
        m["oh"] = np.ascontiguousarray(onehot[sl].astype(b16))
        m["mask"] = np.ascontiguousarray(mask[sl].astype(b16))
        m["ofs"] = np.ascontiguousarray(ofs[sl])
        in_maps.append(m)
    return in_maps


_NC_CACHE = {}


def kernel(**inputs) -> np.ndarray:
    from concourse.bass_utils import run_bass_kernel_spmd
    if T not in _NC_CACHE:
        _NC_CACHE[T] = build_nc(T)
    nc = _NC_CACHE[T]
    in_maps = prep_inputs(**inputs)
    res = run_bass_kernel_spmd(nc, in_maps, list(range(N_CORES)))
    outs = [np.asarray(res.results[c]["out"], np.float32).reshape(ROWS)
            for c in range(N_CORES)]
    return np.concatenate(outs)
